# revision 27
# baseline (speedup 1.0000x reference)
"""Trainium2 Bass kernel for nn_ClauseInferModule (gnn_message_passing).

out[c, b, g] = sum_s prod_l x[b, I[c, g, s, l]],  B=16 G=16384 C=8 S=8 L=4.

Sharding: clause-per-core (C == n_cores == 8). Per core:
  - x arrives host-replicated as [128, G] f32 (partition p = batch p % 16,
    copied for the 8 GPSIMD core groups) and is staged with full-width
    column-split DMAs on the SP + Activation HWDGE queues + Pool SWDGE,
  - GPSIMD ap_gather pulls x[b, idx] for 16 b at once (idx shared across the
    16 partitions of a Q7 core group); each of the 8 groups processes its own
    2048-atom slice of the clause for one substitution s per call,
  - VectorE multiplies the L=4 gathered blocks (step-1 fp32 tensor_tensor)
    and accumulates over the S=8 substitutions,
  - the [128, 2048] accumulator is written wide to HBM; the host
    de-interleaves [16k+b, w] -> [b, k*2048+w].

The int64 index tensor is converted to the int16 "wrapped in 16 partitions"
ap_gather layout on the host (a pure dtype/layout transform).

Structure: raw bacc Block threads (no TileContext) with an explicit
semaphore protocol. The 8 substitutions are merged into 6 ap_gather calls
(each call pays ~1us dispatch + an idx-read ramp of ~0.2us per 1K idx);
the last call is a 4096-idx half so the previous call's vector chain and
the first output half's DMA overlap its gather. x stages on the two HWDGE
queues alone (~26us, HBM-bound); call 0's idx block goes via SWDGE and
the rest stage behind x, needed only ~470us in.

Perf notes (HW-measured): the substitutions' ap_gather calls dominate
(~222us per 65536-idx full call, 27.1ns/idx/Q7-core = 3.39ns/idx
aggregate); the floor is the ~102-cycle reset_reads RD_CMD serving 4
indices in the ap_gather ucode (cayman ReadOverlap=0 - RD_CMDs do not
pipeline), so no caller-side change can beat it. Vector work and idx
staging hide under the gathers.

Alternatives measured on HW, all dead ends:
  - dma_gather (InstDMAGatherAnt, HBM rows, f32 elem 64): 4.6us GPSIMD-busy
    per 1024-idx call on 4 SWDGE queues = 3.33ns/idx aggregate - no better
    than ap_gather (desc-gen is Q7-serial on the same engine, ~2.3cyc/idx).
    Calls >1024 idx (>65 descs/lane) wedge this runtime, and a failed call
    poisons the device for later SWDGE-gather kernels until a kernel with a
    different GPSIMD library runs ("cleansing").
  - indirect_copy (InstIndirectCopy): rejected by neuronxcc walrus
    ("ISA check failed", NCC_IXCG864).
  - On-device x fan-out ([16,G] HBM + 7 SBUF->SBUF copies): slower than
    host-replicated [128,G] staging (~296-346GB/s, already HBM-bound).
"""
import os
import sys
import numpy as np

sys.path.insert(0, "/opt/trn_rl_repo")

import concourse.bacc as bacc
import concourse.bass as bass
from concourse import library_config, mybir
from concourse.bass_utils import run_bass_kernel_spmd

B, G = 16, 16384
C, S, L = 8, 8, 4
NIDX = 8192          # gathers per core group per full ap_gather call
GCHUNK = G // 8      # 2048 target atoms per core group
NQ = 2               # host emits s=S-1 as NQ half-blocks (device merges
                     # the first with s=S-2 into call c4; finer splits lose
                     # more per-call overhead than the shorter tail saves)
QIDX = NIDX // NQ    # idx per half-block
QW = GCHUNK // NQ    # g-window per half-block
XSPLIT = 4096        # x column split: call A gathers only values < XSPLIT

_compiled = None
last_exec_time_ns = None


def _ensure_ntff_hook():
    """Register the axon NTFF profile hook if the antenv stub lacks it."""
    import types
    try:
        from antenv.axon_hooks import get_axon_ntff_profile_hook  # noqa: F401
        return
    except ImportError:
        pass
    try:
        import antenv
        from trn_agent_boot.trn_boot import _ntff_profile_via_ctypes
        mod = types.ModuleType("antenv.axon_hooks")
        _hook = [None]
        mod.set_axon_ntff_profile_hook = lambda h: _hook.__setitem__(0, h)
        mod.get_axon_ntff_profile_hook = lambda: _hook[0]
        sys.modules["antenv.axon_hooks"] = mod
        antenv.axon_hooks = mod
        hook = _ntff_profile_via_ctypes("/opt/axon/libaxon_pjrt.so")
        if hook is not None:
            mod.set_axon_ntff_profile_hook(hook)
    except Exception:
        pass


def _build(m1: int, m2: int):
    # detect_race_conditions=False: the CoreSim race detector has no
    # same-engine program-order model for raw (non-Tile) kernels and flags
    # in-order DVE chains; cross-engine ordering here is explicit via sems.
    nc = bacc.Bacc("TRN2", target_bir_lowering=False, debug=False,
                   detect_race_conditions=False)
    # x stages f32 on the two HWDGE queues in two phases: columns
    # [0, XSPLIT) land first (~2MB, ~14us) so the early sub-call A (whose
    # sorted index values are all < XSPLIT, see _prep_idx) can gather while
    # the remaining 6MB stages. fp16+SWDGE-cast staging measured SLOWER
    # (~150-214GB/s cast path).
    x_d = nc.dram_tensor("x", [128, G], mybir.dt.float32,
                         kind="ExternalInput")
    idx_d = nc.dram_tensor("idx", [128, S * (NIDX // 16)], mybir.dt.int16,
                           kind="ExternalInput")
    # wide output slab; host de-interleaves [16k+b, w] -> [b, k*2048+w]
    out_d = nc.dram_tensor("out", [128, GCHUNK], mybir.dt.float32,
                           kind="ExternalOutput")

    icols = S * (NIDX // 16)
    # Gather-call schedule: ~1us dispatch + idx-read ramp per ap_gather
    # call, so substitutions merge into 6 logical calls (alternating an 8MB
    # and a 4MB buffer); call c0 = s0+s1 additionally splits into A (the
    # m1 sorted low-value indices, gated on phase-1 staging) + B (rest,
    # gated on full staging). s7 splits into two g-halves so the last call
    # is small and its tail overlaps the previous gather.
    #   c0 = s0+s1 (16384 idx = A+B), c1 = s2, c2 = s3+s4, c3 = s5,
    #   c4 = s6 + s7-half0 (12288), c5 = s7-half1 (4096).
    W, H = GCHUNK, GCHUNK // 2
    vcalls = [
        (0,    [(W, 0), (W, 0)]),       # s0, s1
        (1024, [(W, 0)]),               # s2
        (1536, [(W, 0), (W, 0)]),       # s3, s4
        (2560, [(W, 0)]),               # s5
        (3072, [(W, 0), (H, 0)]),       # s6, s7h0 -> acc[0:1024]
        (3840, [(H, H)]),               # s7h1     -> acc[1024:2048]
    ]
    nv = len(vcalls)

    with (
        nc.Block() as block,
        nc.sbuf_tensor("xt", [128, G], mybir.dt.float32) as x_tile,
        nc.sbuf_tensor("it", [128, icols], mybir.dt.int16) as itall,
        nc.sbuf_tensor("g0", [128, 2 * NIDX], mybir.dt.float32) as g0,
        nc.sbuf_tensor("g1", [128, NIDX], mybir.dt.float32) as g1,
        nc.sbuf_tensor("acc", [128, GCHUNK], mybir.dt.float32) as acc,
        nc.sbuf_tensor("tm1", [128, GCHUNK], mybir.dt.float32) as tm1,
        nc.sbuf_tensor("tm2", [128, GCHUNK], mybir.dt.float32) as tm2,
        nc.sbuf_tensor("tm3", [128, GCHUNK], mybir.dt.float32) as tm3,
        nc.semaphore("x1") as sem_x1,
        nc.semaphore("x2") as sem_x2,
        nc.semaphore("stage") as sem_stage,
        nc.semaphore("stageg") as sem_stageg,
        nc.semaphore("itr") as sem_it,
        nc.semaphore("g") as sem_g,
        nc.semaphore("v") as sem_v,
        nc.semaphore("o") as sem_o,
    ):
        gt = (g0, g1)
        XH = XSPLIT // 2

        @block.sync
        def _(sync: bass.BassEngine):
            sync.dma_start(x_tile[:, :XH], x_d[:, :XH]).then_inc(sem_x1, 16)
            sync.dma_start(x_tile[:, XSPLIT:XSPLIT + XH],
                           x_d[:, XSPLIT:XSPLIT + XH]).then_inc(sem_x2, 16)
            sync.dma_start(x_tile[:, 2 * XSPLIT:2 * XSPLIT + (G // 2 - XSPLIT)],
                           x_d[:, 2 * XSPLIT:2 * XSPLIT + (G // 2 - XSPLIT)]
                           ).then_inc(sem_stage, 16)
            sync.dma_start(itall[:, 1024:2560],
                           idx_d[:, 1024:2560]).then_inc(sem_it, 16)
            # half 0 of out (g-windows 0..1023) is final after c4's chain,
            # half 1 after c5's.
            sync.wait_ge(sem_v, nv - 1)
            sync.dma_start(out_d[:, :GCHUNK // 2],
                           acc[:, :GCHUNK // 2]).then_inc(sem_o, 16)
            sync.wait_ge(sem_o, 32)

        @block.scalar
        def _(scalar: bass.BassEngine):
            scalar.dma_start(x_tile[:, XH:XSPLIT],
                             x_d[:, XH:XSPLIT]).then_inc(sem_x1, 16)
            scalar.dma_start(x_tile[:, XSPLIT + XH:2 * XSPLIT],
                             x_d[:, XSPLIT + XH:2 * XSPLIT]
                             ).then_inc(sem_x2, 16)
            scalar.dma_start(x_tile[:, 2 * XSPLIT + (G // 2 - XSPLIT):],
                             x_d[:, 2 * XSPLIT + (G // 2 - XSPLIT):]
                             ).then_inc(sem_stage, 16)
            scalar.dma_start(itall[:, 2560:],
                             idx_d[:, 2560:]).then_inc(sem_it, 16)
            scalar.wait_ge(sem_v, nv)
            scalar.dma_start(out_d[:, GCHUNK // 2:],
                             acc[:, GCHUNK // 2:]).then_inc(sem_o, 16)
            scalar.wait_ge(sem_o, 32)

        @block.gpsimd
        def _(gpsimd: bass.BassGpSimd):
            gpsimd.dma_start(itall[:, :1024],
                             idx_d[:, :1024]).then_inc(sem_stageg, 16)
            gpsimd.load_library(library_config.ap_gather)
            gpsimd.wait_ge(sem_stageg, 16)
            if m1:
                # A: sorted s0/l0 prefix, values < XSPLIT - needs phase 1
                gpsimd.wait_ge(sem_x1, 32)
                gpsimd.ap_gather(g0[:, :m1], x_tile[:, :], itall[:, :m1 // 16],
                                 channels=128, num_elems=G, d=1,
                                 num_idxs=m1).then_inc(sem_g, 1)
            if m2 > m1:
                # A2: values < 2*XSPLIT - needs phases 1+1b
                gpsimd.wait_ge(sem_x2, 32)
                gpsimd.ap_gather(g0[:, m1:m2], x_tile[:, :],
                                 itall[:, m1 // 16:m2 // 16],
                                 channels=128, num_elems=G, d=1,
                                 num_idxs=m2 - m1).then_inc(sem_g, 1)
            # B (rest of c0) and everything later need all of x.
            gpsimd.wait_ge(sem_x1, 32)
            gpsimd.wait_ge(sem_x2, 32)
            gpsimd.wait_ge(sem_stage, 32)
            for j, (coff, segs) in enumerate(vcalls):
                nidx = sum(w for w, _lo in segs) * L
                off = 0
                if j == 0:
                    off = m2
                    nidx -= m2
                    coff = m2 // 16
                if j == 1:
                    gpsimd.wait_ge(sem_it, 32)
                if j >= 2:
                    # WAR: call j reuses gt[j%2]; vector chain j-2 must be
                    # done with it (sem_v counts completed chains).
                    gpsimd.wait_ge(sem_v, j - 1)
                g = gt[j % 2]
                it = itall[:, coff:coff + nidx // 16]
                gpsimd.ap_gather(g[:, off:off + nidx], x_tile[:, :], it[:, :],
                                 channels=128, num_elems=G, d=1,
                                 num_idxs=nidx).then_inc(sem_g, 1)

        @block.vector
        def _(vector: bass.BassEngine):
            first = True
            extra = (1 if m1 else 0) + (1 if m2 > m1 else 0)  # c0 = A+A2+B
            for j, (coff, segs) in enumerate(vcalls):
                g = gt[j % 2]
                vector.wait_ge(sem_g, j + 1 + extra)
                sbase = 0
                for si, (w, lo) in enumerate(segs):
                    last = si == len(segs) - 1

                    def A(l):
                        return g[:, sbase + l * w:sbase + (l + 1) * w]

                    vector.tensor_mul(tm1[:, :w], A(0), A(1))
                    vector.tensor_mul(tm2[:, :w], A(2), A(3))
                    if first:
                        op = vector.tensor_mul(acc[:, :w], tm1[:, :w],
                                               tm2[:, :w])
                        first = False
                    else:
                        vector.tensor_mul(tm3[:, :w], tm1[:, :w], tm2[:, :w])
                        op = vector.tensor_add(acc[:, lo:lo + w],
                                               acc[:, lo:lo + w], tm3[:, :w])
                    if last:
                        op.then_inc(sem_v, 1)
                    sbase += L * w

    nc.compile()
    return nc


def _prep_idx(I: np.ndarray):
    """[C, G, S, L] int64 -> ([C, 128, S*512] int16 wrapped feed, perm, m1).

    Each core group's 2048 tuples are permuted (identically across all s, l)
    so that substitution 0's l=0 indices are ascending. The sorted prefix
    whose values are < 4096 (length >= m1 in every group) can then be
    gathered as soon as x columns [0, 4096) are staged, overlapping the
    rest of the staging. The host un-permutes the output columns.

    Call j covers a g-window of each group's (permuted) tuple slice: stream
    position i = l*(whi-wlo) + (w-wlo) holds I[c, k*2048 + perm[w], s, l];
    ap_gather reads position i of group k from it[16*k + i%16, i//16].
    """
    T = I.astype(np.int16).reshape(C, 8, GCHUNK, S, L)     # [c,k,w,s,l]
    key = T[:, :, :, 0, 0].astype(np.int32)
    perm = np.argsort(key, axis=2, kind="stable")          # [c,k,w]
    T = np.take_along_axis(T, perm[:, :, :, None, None], axis=2)
    key0 = T[:, :, :, 0, 0].astype(np.int32)
    m1 = int((key0 < XSPLIT).sum(axis=2).min())
    m2 = int((key0 < 2 * XSPLIT).sum(axis=2).min())
    # sub-call idx bases need 16-byte alignment (the ucode's uint32 stream
    # reader corrupts every 4th element at misaligned bases), so m1/m2 must
    # be multiples of 128 (= 8 wrapped columns). A2 absorbs at most ~256
    # more early indices - beyond that it just delays B past full staging.
    m1 = (m1 // 128) * 128
    m2 = min((m2 // 128) * 128, m1 + 128)
    if m1 < 256:
        m1, m2 = 0, 0  # degenerate input: skip the early sub-calls
    calls = [(s, 0, GCHUNK) for s in range(S - 1)]
    calls += [(S - 1, q * QW, (q + 1) * QW) for q in range(NQ)]
    blocks = []
    for s, wlo, whi in calls:
        wn = whi - wlo
        st = T[:, :, wlo:whi, s, :]                        # [c,k,w,l]
        st = st.transpose(0, 1, 3, 2).reshape(C, 8, L * wn)  # i = l*wn + w
        wr = st.reshape(C, 8, (L * wn) // 16, 16)          # [c,k,col,pp]
        blocks.append(wr.transpose(0, 1, 3, 2))            # [c,k,pp,col]
    W = np.concatenate(blocks, axis=3)                     # [c,k,pp,allcol]
    W = np.ascontiguousarray(W).reshape(C, 128, S * (NIDX // 16))
    return W, perm, m1, m2


def kernel(x: np.ndarray, I: np.ndarray) -> np.ndarray:
    global _compiled, last_exec_time_ns

    x = np.ascontiguousarray(np.asarray(x), dtype=np.float32)
    xrep = np.ascontiguousarray(np.tile(x, (8, 1)))  # [128, G], p = b%16
    idx_feed, perm, m1, m2 = _prep_idx(np.asarray(I))

    if _compiled is None or _compiled[1] != (m1, m2):
        _compiled = (_build(m1, m2), (m1, m2))
    nc = _compiled[0]

    in_maps = [{"x": xrep, "idx": idx_feed[c]} for c in range(C)]
    kwargs = {}
    if os.environ.get("KERNEL_TRACE") == "1":
        _ensure_ntff_hook()
        kwargs = {"trace": True, "trace_cores": list(range(C))}
    res = run_bass_kernel_spmd(nc, in_maps, core_ids=list(range(C)), **kwargs)
    last_exec_time_ns = res.exec_time_ns
    # wide slab [16k+b, w] -> [b, k*2048 + perm[c,k,w]] (un-permute tuples)
    out = np.empty((C, B, G), dtype=np.float32)
    for c in range(C):
        o = res.results[c]["out"].reshape(8, B, GCHUNK)      # [k, b, w]
        tmp = np.empty_like(o)
        np.put_along_axis(
            tmp, np.broadcast_to(perm[c][:, None, :], o.shape), o, axis=2)
        out[c] = tmp.transpose(1, 0, 2).reshape(B, G)
    return np.ascontiguousarray(out)


if __name__ == "__main__":
    rng = np.random.default_rng(0)
    x = rng.random((B, G), dtype=np.float32)
    I = rng.integers(0, G, size=(C, G, S, L)).astype(np.int64)
    out = kernel(x=x, I=I)
    gathered = x[:, I]
    expect = np.moveaxis(np.sum(np.prod(gathered, axis=-1), axis=-1), 0, 1)
    err = np.abs(out - expect).max() / np.abs(expect).max()
    print("max rel err:", err)


# revision 28
# speedup vs baseline: 1.0014x; 1.0014x over previous
"""Trainium2 Bass kernel for nn_ClauseInferModule (gnn_message_passing).

out[c, b, g] = sum_s prod_l x[b, I[c, g, s, l]],  B=16 G=16384 C=8 S=8 L=4.

Sharding: clause-per-core (C == n_cores == 8). Per core:
  - x arrives host-replicated as [128, G] f32 (partition p = batch p % 16,
    copied for the 8 GPSIMD core groups) and is staged with full-width
    column-split DMAs on the SP + Activation HWDGE queues + Pool SWDGE,
  - GPSIMD ap_gather pulls x[b, idx] for 16 b at once (idx shared across the
    16 partitions of a Q7 core group); each of the 8 groups processes its own
    2048-atom slice of the clause for one substitution s per call,
  - VectorE multiplies the L=4 gathered blocks (step-1 fp32 tensor_tensor)
    and accumulates over the S=8 substitutions,
  - the [128, 2048] accumulator is written wide to HBM; the host
    de-interleaves [16k+b, w] -> [b, k*2048+w].

The int64 index tensor is converted to the int16 "wrapped in 16 partitions"
ap_gather layout on the host (a pure dtype/layout transform).

Structure: raw bacc Block threads (no TileContext) with an explicit
semaphore protocol. The 8 substitutions are merged into 6 ap_gather calls
(each call pays ~1us dispatch + an idx-read ramp of ~0.2us per 1K idx);
the last call is a 4096-idx half so the previous call's vector chain and
the first output half's DMA overlap its gather. x stages on the two HWDGE
queues alone (~26us, HBM-bound); call 0's idx block goes via SWDGE and
the rest stage behind x, needed only ~470us in.

Perf notes (HW-measured): the substitutions' ap_gather calls dominate
(~222us per 65536-idx full call, 27.1ns/idx/Q7-core = 3.39ns/idx
aggregate); the floor is the ~102-cycle reset_reads RD_CMD serving 4
indices in the ap_gather ucode (cayman ReadOverlap=0 - RD_CMDs do not
pipeline), so no caller-side change can beat it. Vector work and idx
staging hide under the gathers.

Alternatives measured on HW, all dead ends:
  - dma_gather (InstDMAGatherAnt, HBM rows, f32 elem 64): 4.6us GPSIMD-busy
    per 1024-idx call on 4 SWDGE queues = 3.33ns/idx aggregate - no better
    than ap_gather (desc-gen is Q7-serial on the same engine, ~2.3cyc/idx).
    Calls >1024 idx (>65 descs/lane) wedge this runtime, and a failed call
    poisons the device for later SWDGE-gather kernels until a kernel with a
    different GPSIMD library runs ("cleansing").
  - indirect_copy (InstIndirectCopy): rejected by neuronxcc walrus
    ("ISA check failed", NCC_IXCG864).
  - On-device x fan-out ([16,G] HBM + 7 SBUF->SBUF copies): slower than
    host-replicated [128,G] staging (~296-346GB/s, already HBM-bound).
"""
import os
import sys
import numpy as np

sys.path.insert(0, "/opt/trn_rl_repo")

import concourse.bacc as bacc
import concourse.bass as bass
from concourse import library_config, mybir
from concourse.bass_utils import run_bass_kernel_spmd

B, G = 16, 16384
C, S, L = 8, 8, 4
NIDX = 8192          # gathers per core group per full ap_gather call
GCHUNK = G // 8      # 2048 target atoms per core group
NQ = 2               # host emits s=S-1 as NQ half-blocks (device merges
                     # the first with s=S-2 into call c4; finer splits lose
                     # more per-call overhead than the shorter tail saves)
QIDX = NIDX // NQ    # idx per half-block
QW = GCHUNK // NQ    # g-window per half-block
XSPLIT = 4096        # x column split: call A gathers only values < XSPLIT

_compiled = None
last_exec_time_ns = None


def _ensure_ntff_hook():
    """Register the axon NTFF profile hook if the antenv stub lacks it."""
    import types
    try:
        from antenv.axon_hooks import get_axon_ntff_profile_hook  # noqa: F401
        return
    except ImportError:
        pass
    try:
        import antenv
        from trn_agent_boot.trn_boot import _ntff_profile_via_ctypes
        mod = types.ModuleType("antenv.axon_hooks")
        _hook = [None]
        mod.set_axon_ntff_profile_hook = lambda h: _hook.__setitem__(0, h)
        mod.get_axon_ntff_profile_hook = lambda: _hook[0]
        sys.modules["antenv.axon_hooks"] = mod
        antenv.axon_hooks = mod
        hook = _ntff_profile_via_ctypes("/opt/axon/libaxon_pjrt.so")
        if hook is not None:
            mod.set_axon_ntff_profile_hook(hook)
    except Exception:
        pass


def _build(m1: int, m2: int):
    # detect_race_conditions=False: the CoreSim race detector has no
    # same-engine program-order model for raw (non-Tile) kernels and flags
    # in-order DVE chains; cross-engine ordering here is explicit via sems.
    nc = bacc.Bacc("TRN2", target_bir_lowering=False, debug=False,
                   detect_race_conditions=False)
    # x stages f32 on the two HWDGE queues in two phases: columns
    # [0, XSPLIT) land first (~2MB, ~14us) so the early sub-call A (whose
    # sorted index values are all < XSPLIT, see _prep_idx) can gather while
    # the remaining 6MB stages. fp16+SWDGE-cast staging measured SLOWER
    # (~150-214GB/s cast path).
    x_d = nc.dram_tensor("x", [128, G], mybir.dt.float32,
                         kind="ExternalInput")
    idx_d = nc.dram_tensor("idx", [128, S * (NIDX // 16)], mybir.dt.int16,
                           kind="ExternalInput")
    # wide output slab; host de-interleaves [16k+b, w] -> [b, k*2048+w]
    out_d = nc.dram_tensor("out", [128, GCHUNK], mybir.dt.float32,
                           kind="ExternalOutput")

    icols = S * (NIDX // 16)
    # Gather-call schedule: ~1us dispatch + idx-read ramp per ap_gather
    # call, so substitutions merge into 6 logical calls (alternating an 8MB
    # and a 4MB buffer); call c0 = s0+s1 additionally splits into A (the
    # m1 sorted low-value indices, gated on phase-1 staging) + B (rest,
    # gated on full staging). s7 splits into two g-halves so the last call
    # is small and its tail overlaps the previous gather.
    #   c0 = s0+s1 (16384 idx = A+B), c1 = s2, c2 = s3+s4, c3 = s5,
    #   c4 = s6 + s7-half0 (12288), c5 = s7-half1 (4096).
    W, H = GCHUNK, GCHUNK // 2
    vcalls = [
        (0,    [(W, 0), (W, 0)]),       # s0, s1
        (1024, [(W, 0)]),               # s2
        (1536, [(W, 0), (W, 0)]),       # s3, s4
        (2560, [(W, 0)]),               # s5
        (3072, [(W, 0), (H, 0)]),       # s6, s7h0 -> acc[0:1024]
        (3840, [(H, H)]),               # s7h1     -> acc[1024:2048]
    ]
    nv = len(vcalls)

    with (
        nc.Block() as block,
        nc.sbuf_tensor("xt", [128, G], mybir.dt.float32) as x_tile,
        nc.sbuf_tensor("it", [128, icols], mybir.dt.int16) as itall,
        nc.sbuf_tensor("g0", [128, 2 * NIDX], mybir.dt.float32) as g0,
        nc.sbuf_tensor("g1", [128, NIDX], mybir.dt.float32) as g1,
        nc.sbuf_tensor("acc", [128, GCHUNK], mybir.dt.float32) as acc,
        nc.sbuf_tensor("tm1", [128, GCHUNK], mybir.dt.float32) as tm1,
        nc.sbuf_tensor("tm2", [128, GCHUNK], mybir.dt.float32) as tm2,
        nc.sbuf_tensor("tm3", [128, GCHUNK], mybir.dt.float32) as tm3,
        nc.semaphore("x1") as sem_x1,
        nc.semaphore("x2") as sem_x2,
        nc.semaphore("stage") as sem_stage,
        nc.semaphore("stageg") as sem_stageg,
        nc.semaphore("itr") as sem_it,
        nc.semaphore("g") as sem_g,
        nc.semaphore("v") as sem_v,
        nc.semaphore("o") as sem_o,
    ):
        gt = (g0, g1)
        XH = XSPLIT // 2

        @block.sync
        def _(sync: bass.BassEngine):
            sync.dma_start(x_tile[:, :XH], x_d[:, :XH]).then_inc(sem_x1, 16)
            sync.dma_start(x_tile[:, XSPLIT:XSPLIT + XH],
                           x_d[:, XSPLIT:XSPLIT + XH]).then_inc(sem_x2, 16)
            sync.dma_start(x_tile[:, 2 * XSPLIT:2 * XSPLIT + (G // 2 - XSPLIT)],
                           x_d[:, 2 * XSPLIT:2 * XSPLIT + (G // 2 - XSPLIT)]
                           ).then_inc(sem_stage, 16)
            sync.dma_start(itall[:, 1024:2560],
                           idx_d[:, 1024:2560]).then_inc(sem_it, 16)
            # half 0 of out (g-windows 0..1023) is final after c4's chain,
            # half 1 after c5's.
            sync.wait_ge(sem_v, nv - 1)
            sync.dma_start(out_d[:, :GCHUNK // 2],
                           acc[:, :GCHUNK // 2]).then_inc(sem_o, 16)
            sync.wait_ge(sem_o, 32)

        @block.scalar
        def _(scalar: bass.BassEngine):
            scalar.dma_start(x_tile[:, XH:XSPLIT],
                             x_d[:, XH:XSPLIT]).then_inc(sem_x1, 16)
            scalar.dma_start(x_tile[:, XSPLIT + XH:2 * XSPLIT],
                             x_d[:, XSPLIT + XH:2 * XSPLIT]
                             ).then_inc(sem_x2, 16)
            scalar.dma_start(x_tile[:, 2 * XSPLIT + (G // 2 - XSPLIT):],
                             x_d[:, 2 * XSPLIT + (G // 2 - XSPLIT):]
                             ).then_inc(sem_stage, 16)
            scalar.dma_start(itall[:, 2560:],
                             idx_d[:, 2560:]).then_inc(sem_it, 16)
            scalar.wait_ge(sem_v, nv)
            scalar.dma_start(out_d[:, GCHUNK // 2:],
                             acc[:, GCHUNK // 2:]).then_inc(sem_o, 16)
            scalar.wait_ge(sem_o, 32)

        @block.gpsimd
        def _(gpsimd: bass.BassGpSimd):
            gpsimd.dma_start(itall[:, :1024],
                             idx_d[:, :1024]).then_inc(sem_stageg, 16)
            gpsimd.load_library(library_config.ap_gather)
            gpsimd.wait_ge(sem_stageg, 16)
            if m1:
                # A: sorted s0/l0 prefix, values < XSPLIT - needs phase 1
                gpsimd.wait_ge(sem_x1, 32)
                gpsimd.ap_gather(g0[:, :m1], x_tile[:, :], itall[:, :m1 // 16],
                                 channels=128, num_elems=G, d=1,
                                 num_idxs=m1).then_inc(sem_g, 1)
            if m2 > m1:
                # A2: values < 2*XSPLIT - needs phases 1+1b
                gpsimd.wait_ge(sem_x2, 32)
                gpsimd.ap_gather(g0[:, m1:m2], x_tile[:, :],
                                 itall[:, m1 // 16:m2 // 16],
                                 channels=128, num_elems=G, d=1,
                                 num_idxs=m2 - m1).then_inc(sem_g, 1)
            # B (rest of c0) and everything later need all of x.
            gpsimd.wait_ge(sem_x1, 32)
            gpsimd.wait_ge(sem_x2, 32)
            gpsimd.wait_ge(sem_stage, 32)
            for j, (coff, segs) in enumerate(vcalls):
                nidx = sum(w for w, _lo in segs) * L
                off = 0
                if j == 0:
                    off = m2
                    nidx -= m2
                    coff = m2 // 16
                if j == 1:
                    gpsimd.wait_ge(sem_it, 32)
                if j >= 2:
                    # WAR: call j reuses gt[j%2]; vector chain j-2 must be
                    # done with it (sem_v counts completed chains).
                    gpsimd.wait_ge(sem_v, j - 1)
                g = gt[j % 2]
                it = itall[:, coff:coff + nidx // 16]
                gpsimd.ap_gather(g[:, off:off + nidx], x_tile[:, :], it[:, :],
                                 channels=128, num_elems=G, d=1,
                                 num_idxs=nidx).then_inc(sem_g, 1)

        @block.vector
        def _(vector: bass.BassEngine):
            first = True
            extra = (1 if m1 else 0) + (1 if m2 > m1 else 0)  # c0 = A+A2+B
            for j, (coff, segs) in enumerate(vcalls):
                g = gt[j % 2]
                vector.wait_ge(sem_g, j + 1 + extra)
                sbase = 0
                for si, (w, lo) in enumerate(segs):
                    last = si == len(segs) - 1

                    def A(l):
                        return g[:, sbase + l * w:sbase + (l + 1) * w]

                    vector.tensor_mul(tm1[:, :w], A(0), A(1))
                    vector.tensor_mul(tm2[:, :w], A(2), A(3))
                    if first:
                        op = vector.tensor_mul(acc[:, :w], tm1[:, :w],
                                               tm2[:, :w])
                        first = False
                    else:
                        vector.tensor_mul(tm3[:, :w], tm1[:, :w], tm2[:, :w])
                        op = vector.tensor_add(acc[:, lo:lo + w],
                                               acc[:, lo:lo + w], tm3[:, :w])
                    if last:
                        op.then_inc(sem_v, 1)
                    sbase += L * w

    nc.compile()
    return nc


def _prep_idx(I: np.ndarray):
    """[C, G, S, L] int64 -> ([C, 128, S*512] int16 wrapped feed, perm, m1).

    Each core group's 2048 tuples are permuted (identically across all s, l)
    so that substitution 0's l=0 indices are ascending. The sorted prefix
    whose values are < 4096 (length >= m1 in every group) can then be
    gathered as soon as x columns [0, 4096) are staged, overlapping the
    rest of the staging. The host un-permutes the output columns.

    Call j covers a g-window of each group's (permuted) tuple slice: stream
    position i = l*(whi-wlo) + (w-wlo) holds I[c, k*2048 + perm[w], s, l];
    ap_gather reads position i of group k from it[16*k + i%16, i//16].
    """
    T = I.astype(np.int16).reshape(C, 8, GCHUNK, S, L)     # [c,k,w,s,l]
    key = T[:, :, :, 0, 0].astype(np.int32)
    perm = np.argsort(key, axis=2, kind="stable")          # [c,k,w]
    T = np.take_along_axis(T, perm[:, :, :, None, None], axis=2)
    key0 = T[:, :, :, 0, 0].astype(np.int32)
    m1 = int((key0 < XSPLIT).sum(axis=2).min())
    m2 = int((key0 < 2 * XSPLIT).sum(axis=2).min())
    # sub-call idx bases need 16-byte alignment (the ucode's uint32 stream
    # reader corrupts every 4th element at misaligned bases), so m1/m2 must
    # be multiples of 128 (= 8 wrapped columns). A2 absorbs at most ~256
    # more early indices - beyond that it just delays B past full staging.
    m1 = (m1 // 128) * 128
    m2 = min((m2 // 128) * 128, m1 + 384)
    if m1 < 256:
        m1, m2 = 0, 0  # degenerate input: skip the early sub-calls
    calls = [(s, 0, GCHUNK) for s in range(S - 1)]
    calls += [(S - 1, q * QW, (q + 1) * QW) for q in range(NQ)]
    blocks = []
    for s, wlo, whi in calls:
        wn = whi - wlo
        st = T[:, :, wlo:whi, s, :]                        # [c,k,w,l]
        st = st.transpose(0, 1, 3, 2).reshape(C, 8, L * wn)  # i = l*wn + w
        wr = st.reshape(C, 8, (L * wn) // 16, 16)          # [c,k,col,pp]
        blocks.append(wr.transpose(0, 1, 3, 2))            # [c,k,pp,col]
    W = np.concatenate(blocks, axis=3)                     # [c,k,pp,allcol]
    W = np.ascontiguousarray(W).reshape(C, 128, S * (NIDX // 16))
    return W, perm, m1, m2


def kernel(x: np.ndarray, I: np.ndarray) -> np.ndarray:
    global _compiled, last_exec_time_ns

    x = np.ascontiguousarray(np.asarray(x), dtype=np.float32)
    xrep = np.ascontiguousarray(np.tile(x, (8, 1)))  # [128, G], p = b%16
    idx_feed, perm, m1, m2 = _prep_idx(np.asarray(I))

    if _compiled is None or _compiled[1] != (m1, m2):
        _compiled = (_build(m1, m2), (m1, m2))
    nc = _compiled[0]

    in_maps = [{"x": xrep, "idx": idx_feed[c]} for c in range(C)]
    kwargs = {}
    if os.environ.get("KERNEL_TRACE") == "1":
        _ensure_ntff_hook()
        kwargs = {"trace": True, "trace_cores": list(range(C))}
    res = run_bass_kernel_spmd(nc, in_maps, core_ids=list(range(C)), **kwargs)
    last_exec_time_ns = res.exec_time_ns
    # wide slab [16k+b, w] -> [b, k*2048 + perm[c,k,w]] (un-permute tuples)
    out = np.empty((C, B, G), dtype=np.float32)
    for c in range(C):
        o = res.results[c]["out"].reshape(8, B, GCHUNK)      # [k, b, w]
        tmp = np.empty_like(o)
        np.put_along_axis(
            tmp, np.broadcast_to(perm[c][:, None, :], o.shape), o, axis=2)
        out[c] = tmp.transpose(1, 0, 2).reshape(B, G)
    return np.ascontiguousarray(out)


if __name__ == "__main__":
    rng = np.random.default_rng(0)
    x = rng.random((B, G), dtype=np.float32)
    I = rng.integers(0, G, size=(C, G, S, L)).astype(np.int64)
    out = kernel(x=x, I=I)
    gathered = x[:, I]
    expect = np.moveaxis(np.sum(np.prod(gathered, axis=-1), axis=-1), 0, 1)
    err = np.abs(out - expect).max() / np.abs(expect).max()
    print("max rel err:", err)
